# revision 1
# baseline (speedup 1.0000x reference)
"""LoRA Linear kernel for Trainium2, 8 NeuronCores, data-parallel over tokens.

out = x @ W^T + bias + 2.0 * (x @ A^T) @ B^T
  x: [4, 2048, 4096] f32, W: [4096, 4096], bias: [4096], A: [16, 4096], B: [4096, 16]

Strategy:
  - Flatten tokens (8192) and shard 1024 tokens per core (pure data parallel,
    no collectives; gather on host).
  - Host pre-transposes so the contraction dim d lands on SBUF partitions:
      xt = x_shard^T [4096, 1024], wt = W^T [4096, 4096], at = A^T [4096, 16].
  - Each core computes out^T [4096, 1024]: for each (o-tile 128, m-chunk 512)
    PSUM tile, accumulate 32 float32r matmuls over d (W stationary), then one
    extra K=128 matmul adds the LoRA update AND the bias:
      ub rows 0..15 = (2*B)^T, row 16 = bias, rows 17..127 = 0
      xab rows 0..15 = xa^T = A x^T, row 16 = ones, rows 17..127 = 0
  - float32r matmuls run at 1 cycle/row for N>=256 (4x faster than float32).
  - Host transposes/concats the 8 out^T shards back to [4, 2048, 4096].
"""

import sys
from contextlib import ExitStack

import numpy as np

sys.path.insert(0, "/opt/trn_rl_repo")

import concourse.bacc as bacc  # noqa: E402
import concourse.bass as bass  # noqa: E402
import concourse.mybir as mybir  # noqa: E402
import concourse.tile as tile  # noqa: E402
from concourse.bass import ts  # noqa: E402
from concourse.bass_utils import run_bass_kernel_spmd  # noqa: E402

P = 128
B_DIM, S_DIM = 4, 2048
D = 4096          # in_features (contraction)
O = 4096          # out_features
R = 16            # lora rank
SCALING = 2.0     # alpha / rank = 32/16
NCORES = 8
M = (B_DIM * S_DIM) // NCORES   # tokens per core = 1024
KD = D // P       # 32 contraction tiles
MC = 512          # moving free dim per matmul
NMC = M // MC     # 2 m-chunks
NO = O // P       # 32 output-feature tiles

FR = mybir.dt.float32r
F32 = mybir.dt.float32


def build_program() -> bass.Bass:
    # Bacc (not plain Bass): its compile() pipeline splits multi-wait
    # matmuls via event semaphores — walrus allows at most one sync wait
    # on a self-loading f32r matmul.
    nc = bacc.Bacc()
    xt = nc.dram_tensor("xt", [D, M], FR, kind="ExternalInput")
    wt = nc.dram_tensor("wt", [D, O], FR, kind="ExternalInput")
    at = nc.dram_tensor("at", [D, R], FR, kind="ExternalInput")
    # ub: rows 0..15 = (2*lora_b)^T, row 16 = bias, rows 17..127 = 0
    ub = nc.dram_tensor("ub", [P, O], FR, kind="ExternalInput")
    # fill for xab rows 16..127: row 16 = ones, rest zeros
    fill = nc.dram_tensor("fill", [P - R, NMC, MC], FR, kind="ExternalInput")
    outT = nc.dram_tensor("outT", [O, M], F32, kind="ExternalOutput")

    xt_r = xt.rearrange("(ko p) m -> p ko m", p=P)   # [128, 32, 1024]
    at_r = at.rearrange("(ko p) r -> p ko r", p=P)   # [128, 32, 16]
    wt_r = wt.rearrange("(ko p) o -> p ko o", p=P)   # [128, 32, 4096]

    with ExitStack() as ctx:
        tc = ctx.enter_context(tile.TileContext(nc))
        xt_pool = ctx.enter_context(tc.tile_pool(name="xtp", bufs=1))
        cpool = ctx.enter_context(tc.tile_pool(name="cpool", bufs=1))
        wt_pool = ctx.enter_context(tc.tile_pool(name="wtp", bufs=2))
        out_pool = ctx.enter_context(tc.tile_pool(name="outp", bufs=4))
        ps_pool = ctx.enter_context(tc.tile_pool(name="psp", bufs=4, space="PSUM"))
        psxa_pool = ctx.enter_context(tc.tile_pool(name="psxa", bufs=2, space="PSUM"))

        xt_sb = xt_pool.tile([P, KD, M], FR)
        at_sb = cpool.tile([P, KD, R], FR)
        ub_sb = cpool.tile([P, O], FR)            # rows 0..16 real, rest zero
        xab_sb = cpool.tile([P, NMC, MC], FR)     # rows 0..16 real, rest zero

        # Constant loads on the ACT HWDGE ring; weight stream on the SP ring.
        # (memset can't produce f32r, so all constant fills come via DMA.)
        nc.scalar.dma_start(at_sb[:], at_r)
        nc.scalar.dma_start(ub_sb[:], ub[:])
        nc.scalar.dma_start(xab_sb[R:P, :, :], fill[:])
        # Split the x^T load along the contraction dim so the first matmuls
        # (which consume ko-chunk 0) start ~18us earlier than one bulk DMA.
        XSPLIT = 4
        kchunk = KD // XSPLIT
        for mi in range(NMC):
            for h in range(XSPLIT):
                nc.scalar.dma_start(
                    xt_sb[:, ts(h, kchunk), ts(mi, MC)],
                    xt_r[:, ts(h, kchunk), ts(mi, MC)],
                )

        # xa^T[r, m] = sum_d A^T[d, r]^T x^T[d, m] for each m-chunk
        for mi in range(NMC):
            ps_xa = psxa_pool.tile([R, MC], F32)
            for k in range(KD):
                nc.tensor.matmul(
                    ps_xa[:],
                    lhsT=at_sb[:, k, :],
                    rhs=xt_sb[:, k, ts(mi, MC)],
                    start=(k == 0),
                    stop=(k == KD - 1),
                )
            # out dtype f32r => DVE rounds to the 20-bit fp32r format, as the
            # BIR verifier requires for matmul operands.
            nc.vector.tensor_copy(out=xab_sb[0:R, mi, :], in_=ps_xa[:])

        # Main: out^T tile [o=128, m=512] = W-block^T.T @ x^T + ub^T.T @ xab
        for oi in range(NO):
            wt_sb = wt_pool.tile([P, KD, P], FR)
            nc.sync.dma_start(wt_sb[:], wt_r[:, :, ts(oi, P)])
            for mi in range(NMC):
                ps = ps_pool.tile([P, MC], F32)
                for k in range(KD):
                    nc.tensor.matmul(
                        ps[:],
                        lhsT=wt_sb[:, k, :],
                        rhs=xt_sb[:, k, ts(mi, MC)],
                        start=(k == 0),
                        stop=False,
                    )
                # rank-16 LoRA update + bias (via the ones row), zero-padded to K=128
                nc.tensor.matmul(
                    ps[:],
                    lhsT=ub_sb[:, ts(oi, P)],
                    rhs=xab_sb[:, mi, :],
                    start=False,
                    stop=True,
                )
                ot = out_pool.tile([P, MC], F32)
                nc.vector.tensor_copy(out=ot[:], in_=ps[:])
                nc.gpsimd.dma_start(outT[ts(oi, P), ts(mi, MC)], ot[:])
    nc.compile()
    return nc


def round_f32r(a: np.ndarray) -> np.ndarray:
    """Round-to-nearest-even into the fp32r format (1s + 8e + 11m, low 12
    bits zero). Matmul operands must be pre-rounded for well-defined HW
    behavior; the on-chip producers round, so round host inputs too."""
    b = np.ascontiguousarray(a, dtype=np.float32).view(np.uint32)
    lsb = (b >> np.uint32(12)) & np.uint32(1)
    r = (b + np.uint32(0x07FF) + lsb) & np.uint32(0xFFFFF000)
    return r.view(np.float32)


def prepare_in_maps(inputs, weight, bias, lora_a, lora_b):
    x = round_f32r(
        np.ascontiguousarray(np.asarray(inputs, dtype=np.float32)).reshape(
            B_DIM * S_DIM, D
        )
    )
    wt = round_f32r(np.ascontiguousarray(np.asarray(weight, dtype=np.float32).T))
    at = round_f32r(np.ascontiguousarray(np.asarray(lora_a, dtype=np.float32).T))
    ub = round_f32r(
        np.concatenate(
            [
                SCALING * np.asarray(lora_b, dtype=np.float32).T,
                np.asarray(bias, dtype=np.float32)[None, :],
                np.zeros((P - R - 1, O), dtype=np.float32),
            ],
            axis=0,
        ).astype(np.float32)
    )
    fill = np.zeros((P - R, NMC, MC), dtype=np.float32)
    fill[0] = 1.0
    in_maps = []
    for c in range(NCORES):
        xt_c = np.ascontiguousarray(x[c * M : (c + 1) * M].T)
        in_maps.append({"xt": xt_c, "wt": wt, "at": at, "ub": ub, "fill": fill})
    return in_maps


def run(inputs, weight, bias, lora_a, lora_b, trace=False):
    nc = build_program()
    in_maps = prepare_in_maps(inputs, weight, bias, lora_a, lora_b)
    res = run_bass_kernel_spmd(nc, in_maps, list(range(NCORES)), trace=trace)
    shards = [np.asarray(res.results[c]["outT"]).T for c in range(NCORES)]
    out = np.concatenate(shards, axis=0).reshape(B_DIM, S_DIM, O)
    return np.ascontiguousarray(out, dtype=np.float32), res


def kernel(inputs, weight, bias, lora_a, lora_b):
    out, _ = run(inputs, weight, bias, lora_a, lora_b, trace=False)
    return out

